# revision 1
# baseline (speedup 1.0000x reference)
# Trainium2 Bass kernel for ChannelMambaFusionBlock (2-modal channel-scan
# mamba fusion block).  Self-contained: takes FULL inputs, shards across 8
# NeuronCores internally (data-parallel over batch x H-rows), returns FULL
# outputs (y_r, y_e).
#
# Sharding: core i handles batch b = i//4, H-rows [32*(i%4), 32*(i%4)+32).
# Each core receives a 34-row halo slab of both input images (bf16, x-padded
# to 130 cols) plus replicated weights; all intermediates stay in SBUF.
#
# Per-core pipeline:
#   A. channel-LN       positions-on-partitions via PE transpose; stats on ACT
#   B. gconv 1x1        dense 96->192 matmul (block-diag weights baked in)
#      dwconv 3x3       9 diagonal accumulating matmuls over the padded slab
#      silu             exp on ACT + divide on DVE (single ACT table set)
#   C. xproj/dtproj     block-diag matmuls contracting the head dim
#   D. channel scan     per H-row, per modality: [128pos, (d,n,ci)=2304] tiles
#                         dA = exp(dt*A)  (TT bcast + ACT exp)
#                         w  = dt*u*B     (TT bcast)
#                         h  = tensor_tensor_scan(dA, w) along ci
#                         y  = sum_n h*C  (TT bcast + f32 add tree)
#   E. head-LN, outproj, residual, channel-LN
#   F. MLP (exact gelu) + residual  (separate ACT table set, ordered last)

import numpy as np
import ml_dtypes

HEAD, NST, CI = 16, 12, 12       # heads, d_state, d_inner per head (scan len)
B, C, H, W = 2, 96, 128, 128
G, E = 24, 192
ROWS = 32                        # H-rows per core
SLAB_R = ROWS + 2                # with halo rows
WPAD = W + 2                     # x-padded row width
SLAB = SLAB_R * WPAD             # 4420
P = ROWS * W                     # 4096 inner positions per core
NCORES = 8

_bf16 = ml_dtypes.bfloat16


def _eprime(c, d):
    """E' channel ordering used on-chip: e' = c*16 + d (c-major)."""
    return c * HEAD + d


def _eorig(c, d):
    return d * CI + c


# channel-tile q = k*2+h holds mod k's E'-channels [h*96, (h+1)*96)
def _tile_chan(q, j):
    return q // 2, (q % 2) * 96 + j


def _prep_consts(inp):
    f32 = np.float32
    out = {}
    mods = ['rgb', 'e']

    out['ident'] = np.eye(128, dtype=f32)

    # gconv lhsT [96, 2*192]: cols k*192+e', rows = input channel
    wi = np.zeros((96, 2 * 192), f32)
    for k, m in enumerate(mods):
        ip = np.asarray(inp[f'inproj_{m}_w'], f32)                 # (24,8,4)
        Wd = np.zeros((E, C), f32)
        for g in range(G):
            Wd[g * 8:(g + 1) * 8, g * 4:(g + 1) * 4] = ip[g]
        for cc in range(CI):
            for d in range(HEAD):
                wi[:, k * 192 + _eprime(cc, d)] = Wd[_eorig(cc, d), :]
    out['wi'] = wi

    # dwconv diag lhsT [128, 3*9*128] (tile t, tap): cols (t*9+tap)*128 + m
    convw = [np.asarray(inp[f'conv_{m}_w'], f32)[:, 0] for m in mods]  # (192,3,3)
    convb = [np.asarray(inp[f'conv_{m}_b'], f32) for m in mods]
    ep2eo = np.zeros(E, np.int32)
    for cc in range(CI):
        for d in range(HEAD):
            ep2eo[_eprime(cc, d)] = _eorig(cc, d)
    wdw = np.zeros((96, 36 * 96), f32)
    cb = np.zeros((96, 4), f32)
    for q in range(4):
        for j in range(96):
            k, ep = _tile_chan(q, j)
            eo = ep2eo[ep]
            cb[j, q] = convb[k][eo]
            for tap in range(9):
                dy, dx = tap // 3, tap % 3
                wdw[j, (q * 9 + tap) * 96 + j] = convw[k][eo, dy, dx]
    out['wdw'] = wdw
    out['cb'] = cb

    # xproj lhsTs.  K-rows = (c_loc, d) = c_loc*16+d (c_loc in half).
    #   xp1 [96, 4*78]: per (k,h): cols 0..71 = B rows (n, c_loc) n-major,
    #                   cols 72..77 = dts rows.     xp2 [96, 4*72]: C rows.
    xpw = np.asarray(inp['xproj_w'], f32)                          # (2,25,16)
    xp1 = np.zeros((96, 4 * 78), f32)
    xp2 = np.zeros((96, 4 * 72), f32)
    for k in range(2):
        for h in range(2):
            o1, o2 = (k * 2 + h) * 78, (k * 2 + h) * 72
            for cl in range(6):
                for d in range(HEAD):
                    row = cl * HEAD + d
                    for n in range(NST):
                        xp1[row, o1 + n * 6 + cl] = xpw[k, 1 + n, d]
                        xp2[row, o2 + n * 6 + cl] = xpw[k, 13 + n, d]
                    xp1[row, o1 + 72 + cl] = xpw[k, 0, d]
    out['xp1'], out['xp2'] = xp1, xp2

    rep = lambda v: np.broadcast_to(np.asarray(v, f32).reshape(1, -1),
                                    (128, np.asarray(v).size)).copy()
    out['ga'] = rep(np.stack([np.asarray(inp['in1_w']), np.asarray(inp['in2_w'])]))
    out['be'] = rep(np.stack([np.asarray(inp['in1_b']), np.asarray(inp['in2_b'])]))
    out['nw'] = rep(np.stack([np.asarray(inp['n1_w']), np.asarray(inp['n2_w'])]))
    out['nb'] = rep(np.stack([np.asarray(inp['n1_b']), np.asarray(inp['n2_b'])]))
    A = -np.exp(np.asarray(inp['A_logs'], f32))                    # (2,16,12)
    out['A_rep'] = rep(A)                                          # [128, 384]
    out['dtw_rep'] = rep(np.asarray(inp['dtproj_w'], f32)[:, :, 0])  # [128,32]
    out['dtb_rep'] = rep(np.asarray(inp['dtproj_b'], f32))           # f32
    out['Ds_rep'] = rep(np.asarray(inp['Ds'], f32))
    out['onw'] = rep(np.stack([np.asarray(inp['on1_w']), np.asarray(inp['on2_w'])]))
    out['onb'] = rep(np.stack([np.asarray(inp['on1_b']), np.asarray(inp['on2_b'])]))

    # outproj lhsT [96, 4*96]: per (k, half): rows = e_orig in half, cols = C
    wo = np.zeros((96, 4 * 96), f32)
    for k, m in enumerate(mods):
        op = np.asarray(inp[f'outproj_{m}_w'], f32)                # (24,4,8)
        Wd = np.zeros((C, E), f32)
        for g in range(G):
            Wd[g * 4:(g + 1) * 4, g * 8:(g + 1) * 8] = op[g]
        for h in range(2):
            wo[:, (k * 2 + h) * 96:(k * 2 + h + 1) * 96] = \
                Wd[:, h * 96:(h + 1) * 96].T
    out['wo'] = wo

    # mlp
    w1 = np.zeros((96, 2 * 384), f32)
    w2 = np.zeros((128, 6 * 96), f32)
    b1 = np.zeros((128, 6), f32)
    b2 = np.zeros((96, 2), f32)
    for k, m in enumerate(mods):
        w1[:, k * 384:(k + 1) * 384] = np.asarray(inp[f'mlp_{m}_fc1_w'], f32).T
        b1k = np.asarray(inp[f'mlp_{m}_fc1_b'], f32)
        w2k = np.asarray(inp[f'mlp_{m}_fc2_w'], f32)               # (96, 384)
        for j in range(3):
            w2[:, (k * 3 + j) * 96:(k * 3 + j + 1) * 96] = \
                w2k[:, j * 128:(j + 1) * 128].T
            b1[:, k * 3 + j] = b1k[j * 128:(j + 1) * 128]
        b2[:, k] = np.asarray(inp[f'mlp_{m}_fc2_b'], f32)
    out['w1'], out['w2'], out['b1'], out['b2'] = w1, w2, b1, b2
    return out


def _core_slabs(inp, core):
    b, r0 = core // 4, (core % 4) * ROWS
    res = {}
    for name, key in (('x_r', 'x_rgb'), ('x_e', 'x_e')):
        x = np.asarray(inp[key], np.float32)
        s = np.zeros((C, SLAB_R, WPAD), np.float32)
        lo, hi = r0 - 1, r0 + ROWS + 1
        slo, shi = max(lo, 0), min(hi, H)
        s[:, (slo - lo):(shi - lo), 1:W + 1] = x[b, :, slo:shi, :]
        res[name] = np.ascontiguousarray(s.reshape(C, SLAB)).astype(_bf16)
    return res


_F32_CONSTS = ('cb', 'dtb_rep', 'b1', 'b2')


def _build_program(sim_safe=False):
    import concourse.mybir as mybir
    from concourse import bacc
    import concourse.tile as tile
    from concourse.tile import add_dep_helper
    from contextlib import ExitStack

    bf16 = mybir.dt.bfloat16
    f32 = mybir.dt.float32
    Alu = mybir.AluOpType
    AF = mybir.ActivationFunctionType
    AX = mybir.AxisListType

    nc = bacc.Bacc("TRN2", target_bir_lowering=False, debug=False)

    shapes = {
        'x_r': (C, SLAB), 'x_e': (C, SLAB), 'ident': (128, 128),
        'wi': (96, 384), 'wdw': (96, 36 * 96), 'cb': (96, 4),
        'xp1': (96, 312), 'xp2': (96, 288),
        'ga': (128, 192), 'be': (128, 192), 'nw': (128, 192), 'nb': (128, 192),
        'A_rep': (128, 384), 'dtw_rep': (128, 32), 'dtb_rep': (128, 32),
        'Ds_rep': (128, 32), 'onw': (128, 24), 'onb': (128, 24),
        'wo': (96, 384), 'w1': (96, 768), 'w2': (128, 576),
        'b1': (128, 6), 'b2': (96, 2),
    }
    dram = {}
    for name, shp in shapes.items():
        dt = f32 if name in _F32_CONSTS else bf16
        dram[name] = nc.dram_tensor(name, list(shp), dt,
                                    kind="ExternalInput").ap()
    y_out = {0: nc.dram_tensor('y_r', [C, P], f32, kind="ExternalOutput").ap(),
             1: nc.dram_tensor('y_e', [C, P], f32, kind="ExternalOutput").ap()}

    with tile.TileContext(nc) as tc, ExitStack() as ctx:
        slab = ctx.enter_context(tc.tile_pool(name="slab", bufs=1))
        cst = ctx.enter_context(tc.tile_pool(name="cst", bufs=1))
        big = ctx.enter_context(tc.tile_pool(name="big", bufs=1))
        work = ctx.enter_context(tc.tile_pool(name="work", bufs=2))
        ps_tr = ctx.enter_context(tc.tile_pool(name="ps_tr", bufs=2, space="PSUM"))
        ps_mm = ctx.enter_context(tc.tile_pool(name="ps_mm", bufs=4, space="PSUM"))
        ps_m2 = ctx.enter_context(tc.tile_pool(name="ps_m2", bufs=2, space="PSUM"))

        cs = {}
        for name in shapes:
            if name in ('x_r', 'x_e'):
                continue
            dt = f32 if name in _F32_CONSTS else bf16
            cs[name] = cst.tile(list(shapes[name]), dt, name=f"c_{name}")
            nc.sync.dma_start(cs[name][:], dram[name])
        ident = cs['ident']
        negcb = cst.tile([96, 4], f32, name="negcb")
        nc.vector.tensor_scalar(negcb[:], cs['cb'][:], -1.0, None, Alu.mult)
        eps6 = cst.tile([128, 1], f32, name="eps6")
        nc.vector.memset(eps6[:], 1e-6)
        eps5 = cst.tile([128, 1], f32, name="eps5")
        nc.vector.memset(eps5[:], 1e-5)
        if sim_safe:
            b1s = cst.tile([128, 6], f32, name="b1s")
            nc.vector.tensor_scalar(b1s[:], cs['b1'][:], 1.702, None, Alu.mult)

        xs = {0: slab.tile([C, SLAB], bf16, name="xs_r"),
              1: slab.tile([C, SLAB], bf16, name="xs_e")}
        nc.sync.dma_start(xs[0][:], dram['x_r'])
        nc.sync.dma_start(xs[1][:], dram['x_e'])

        # ================= stage A: channel LN (eps 1e-6) =================
        xr = {0: slab.tile([C, SLAB], bf16, name="xr_r"),
              1: slab.tile([C, SLAB], bf16, name="xr_e")}
        nchunkA = (SLAB + 127) // 128
        for k in range(2):
            for j in range(nchunkA):
                p0 = j * 128
                npos = min(128, SLAB - p0)
                pt = ps_tr.tile([128, 96], bf16, name="lnA_ps", tag="ptr")
                nc.tensor.matmul(pt[:npos, :], xs[k][:, p0:p0 + npos],
                                 ident[:96, :96], is_transpose=True)
                xt = work.tile([128, 96], bf16, name="lnA_xt")
                s1 = work.tile([128, 1], f32, name="lnA_s1")
                nc.scalar.activation(xt[:npos], pt[:npos, :], AF.Copy,
                                     accum_out=s1[:npos])
                sq = work.tile([128, 96], bf16, name="lnA_sq")
                s2 = work.tile([128, 1], f32, name="lnA_s2")
                nc.scalar.activation(sq[:npos], xt[:npos], AF.Square,
                                     accum_out=s2[:npos])
                mu = work.tile([128, 1], f32, name="lnA_mu")
                nc.vector.tensor_scalar(mu[:npos], s1[:npos], 1.0 / 96, None,
                                        Alu.mult)
                m2 = work.tile([128, 1], f32, name="lnA_m2")
                nc.vector.tensor_tensor(m2[:npos], mu[:npos], mu[:npos],
                                        Alu.mult)
                var = work.tile([128, 1], f32, name="lnA_var")
                nc.vector.scalar_tensor_tensor(var[:npos], s2[:npos], 1.0 / 96,
                                               m2[:npos], Alu.mult,
                                               Alu.subtract)
                lnv = work.tile([128, 1], f32, name="lnA_lnv")
                nc.scalar.activation(lnv[:npos], var[:npos], AF.Ln, bias=eps6[:npos])
                rstd = work.tile([128, 1], f32, name="lnA_rstd")
                nc.scalar.activation(rstd[:npos], lnv[:npos], AF.Exp,
                                     scale=-0.5)
                xn = work.tile([128, 96], bf16, name="lnA_xn")
                nc.vector.tensor_scalar(xn[:npos], xt[:npos], mu[:npos],
                                        rstd[:npos], Alu.subtract, Alu.mult)
                xa = work.tile([128, 96], bf16, name="lnA_xa")
                nc.vector.tensor_tensor(xa[:npos], xn[:npos],
                                        cs['ga'][:npos, k * 96:(k + 1) * 96],
                                        Alu.mult)
                xb_ = work.tile([128, 96], bf16, name="lnA_xb")
                nc.vector.tensor_tensor(xb_[:npos], xa[:npos],
                                        cs['be'][:npos, k * 96:(k + 1) * 96],
                                        Alu.add)
                pb = ps_tr.tile([96, 128], bf16, name="lnA_pb", tag="ptr")
                nc.tensor.matmul(pb[:, :npos], xb_[:npos, :],
                                 ident[:npos, :npos], is_transpose=True)
                nc.scalar.activation(xr[k][:, p0:p0 + npos], pb[:, :npos],
                                     AF.Copy)

        # ================= stage B: gconv =================
        z = {q: slab.tile([96, SLAB], bf16, name=f"z{q}") for q in range(4)}
        CH = 512
        nchunkB = (SLAB + CH - 1) // CH
        for j in range(nchunkB):
            p0 = j * CH
            npos = min(CH, SLAB - p0)
            for q in range(4):
                k, h = q // 2, q % 2
                pz = ps_mm.tile([96, CH], f32, name="gc_ps", tag="pmm")
                nc.tensor.matmul(pz[:, :npos],
                                 cs['wi'][:, k * 192 + h * 96:
                                          k * 192 + (h + 1) * 96],
                                 xr[k][:, p0:p0 + npos])
                nc.scalar.activation(z[q][:, p0:p0 + npos], pz[:, :npos],
                                     AF.Copy)

        # ============ stage B2: dwconv (9 diag taps) + bias + silu ==========
        u = {q: slab.tile([96, SLAB], bf16, name=f"u{q}") for q in range(4)}
        for q in range(4):
            for st in range(131, 4289, 512):
                npos = min(512, 4289 - st)
                pu = ps_mm.tile([96, 512], f32, name="dw_ps", tag="pmm")
                for tap in range(9):
                    dy, dx = tap // 3, tap % 3
                    dlt = (dy - 1) * WPAD + (dx - 1)
                    nc.tensor.matmul(
                        pu[:, :npos],
                        cs['wdw'][:, (q * 9 + tap) * 96:(q * 9 + tap + 1) * 96],
                        z[q][:, st + dlt: st + dlt + npos],
                        start=(tap == 0), stop=(tap == 8))
                # silu(v) = v / (1 + exp(-v)), v = psum + cb
                ex = work.tile([96, 512], bf16, name="dw_ex")
                nc.scalar.activation(ex[:, :npos], pu[:, :npos], AF.Exp,
                                     scale=-1.0, bias=negcb[:, q:q + 1])
                xpre = work.tile([96, 512], bf16, name="dw_xpre")
                nc.scalar.activation(xpre[:, :npos], pu[:, :npos], AF.Identity,
                                     bias=cs['cb'][:, q:q + 1])
                den = work.tile([96, 512], bf16, name="dw_den")
                nc.vector.tensor_scalar(den[:, :npos], ex[:, :npos], 1.0, None,
                                        Alu.add)
                rcp = work.tile([96, 512], bf16, name="dw_rcp")
                with nc.allow_low_precision(reason="silu sigmoid recip"):
                    nc.vector.reciprocal(rcp[:, :npos], den[:, :npos])
                nc.vector.tensor_tensor(u[q][:, st:st + npos], xpre[:, :npos],
                                        rcp[:, :npos], Alu.mult)

        # ============ stages C/D/E: per-row scan pipeline ============
        z1s = {0: slab.tile([96, P], bf16, name="z1_r"),
               1: slab.tile([96, P], bf16, name="z1_e")}
        zns = {0: slab.tile([96, P], bf16, name="zn_r", tag="xs_r"),
               1: slab.tile([96, P], bf16, name="zn_e", tag="xs_e")}
        last_exp = [None]
        first_gelu = [None]

        for r_ in range(ROWS):
            p0 = r_ * W
            sp0 = (r_ + 1) * WPAD + 1
            # ---- xproj for this row ----
            xbr = {}
            xcr = {}
            for k in range(2):
                for h in range(2):
                    pb_ = ps_m2.tile([78, 128], f32, name="xp_ps1", tag="pm2")
                    pc_ = ps_m2.tile([72, 128], f32, name="xp_ps2", tag="pm2")
                    o1, o2 = (k * 2 + h) * 78, (k * 2 + h) * 72
                    rhs = u[k * 2 + h][:, sp0:sp0 + W]
                    nc.tensor.matmul(pb_[:, :], cs['xp1'][:, o1:o1 + 78], rhs)
                    nc.tensor.matmul(pc_[:, :], cs['xp2'][:, o2:o2 + 72], rhs)
                    xbr[(k, h)] = work.tile([78, 128], bf16, name=f"xbr{k}{h}")
                    xcr[(k, h)] = work.tile([72, 128], bf16, name=f"xcr{k}{h}")
                    nc.scalar.activation(xbr[(k, h)][:], pb_[:, :], AF.Copy)
                    nc.scalar.activation(xcr[(k, h)][:], pc_[:, :], AF.Copy)

            # ---- transposes to pos layout ----
            dts_t = {k: work.tile([128, 12], bf16, name=f"sc_dts{k}")
                     for k in range(2)}
            B_t = {k: work.tile([128, 144], bf16, name=f"sc_B{k}")
                   for k in range(2)}
            C_t = {k: work.tile([128, 144], bf16, name=f"sc_C{k}")
                   for k in range(2)}
            u_t = {k: work.tile([128, 192], bf16, name=f"sc_u{k}")
                   for k in range(2)}
            for k in range(2):
                for h in range(2):
                    ptb = ps_tr.tile([128, 78], bf16, name="sc_psb", tag="ptr")
                    nc.tensor.matmul(ptb[:, :], xbr[(k, h)][:, :],
                                     ident[:78, :78], is_transpose=True)
                    bdst = B_t[k][:].rearrange("p (n c) -> p n c", n=NST)
                    nc.scalar.activation(
                        bdst[:, :, h * 6:h * 6 + 6],
                        ptb[:, 0:72].rearrange("p (n c) -> p n c", n=NST),
                        AF.Copy)
                    nc.scalar.activation(dts_t[k][:, h * 6:h * 6 + 6],
                                         ptb[:, 72:78], AF.Copy)
                    ptc = ps_tr.tile([128, 72], bf16, name="sc_psc", tag="ptr")
                    nc.tensor.matmul(ptc[:, :], xcr[(k, h)][:, :],
                                     ident[:72, :72], is_transpose=True)
                    cdst = C_t[1 - k][:].rearrange("p (n c) -> p n c", n=NST)
                    nc.scalar.activation(
                        cdst[:, :, h * 6:h * 6 + 6],
                        ptc[:, :].rearrange("p (n c) -> p n c", n=NST),
                        AF.Copy)
                for h in range(2):
                    ptu = ps_tr.tile([128, 96], bf16, name="sc_psu", tag="ptr")
                    nc.tensor.matmul(ptu[:, :], u[k * 2 + h][:, sp0:sp0 + W],
                                     ident[:96, :96], is_transpose=True)
                    nc.scalar.activation(u_t[k][:, h * 96:(h + 1) * 96],
                                         ptu[:, :], AF.Copy)

            # ---- per-modality scan core ----
            for k in range(2):
                kd = slice(k * 16, (k + 1) * 16)
                # dt = softplus(dts*dtw + dtb)   [(d,ci) = 192]
                pre = work.tile([128, 192], bf16, name="sc_pre")
                pre3 = pre[:].rearrange("p (d c) -> p d c", d=HEAD)
                nc.vector.tensor_tensor(
                    pre3,
                    dts_t[k][:].unsqueeze(1).broadcast_to([128, HEAD, CI]),
                    cs['dtw_rep'][:, kd].unsqueeze(2).broadcast_to(
                        [128, HEAD, CI]), Alu.mult)
                pre2 = work.tile([128, 192], bf16, name="sc_pre2")
                nc.vector.tensor_tensor(
                    pre2[:].rearrange("p (d c) -> p d c", d=HEAD), pre3,
                    cs['dtb_rep'][:, kd].unsqueeze(2).broadcast_to(
                        [128, HEAD, CI]), Alu.add)
                ab = work.tile([128, 192], bf16, name="sc_ab")
                nc.scalar.activation(ab[:], pre2[:], AF.Abs)
                exn = work.tile([128, 192], bf16, name="sc_exn")
                nc.scalar.activation(exn[:], ab[:], AF.Exp, scale=-1.0)
                lg = work.tile([128, 192], bf16, name="sc_lg")
                nc.scalar.activation(lg[:], exn[:], AF.Ln, bias=1.0)
                rl = work.tile([128, 192], bf16, name="sc_rl")
                nc.vector.tensor_scalar(rl[:], pre2[:], 0.0, None, Alu.max)
                dt = work.tile([128, 192], bf16, name="sc_dt")
                nc.vector.tensor_tensor(dt[:], rl[:], lg[:], Alu.add)
                dt3 = dt[:].rearrange("p (d c) -> p d c", d=HEAD)

                # dtu  [(d,ci)]; u_t is (c,d) -> strided view
                u_dc = u_t[k][:].rearrange("p (c d) -> p d c", c=CI)
                dtu = work.tile([128, 192], bf16, name="sc_dtu")
                nc.vector.tensor_tensor(
                    dtu[:].rearrange("p (d c) -> p d c", d=HEAD), dt3, u_dc,
                    Alu.mult)

                # dA = exp(dt * A); ci=0 forced to ~0
                arg = big.tile([128, 2304], bf16, name="sc_arg")
                arg4 = arg[:].rearrange("p (d n c) -> p d n c", d=HEAD, n=NST)
                A3 = cs['A_rep'][:, k * 192:(k + 1) * 192].rearrange(
                    "p (d n) -> p d n", d=HEAD)
                nc.vector.tensor_tensor(
                    arg4,
                    dt3.unsqueeze(2).broadcast_to([128, HEAD, NST, CI]),
                    A3.unsqueeze(3).broadcast_to([128, HEAD, NST, CI]),
                    Alu.mult)
                nc.vector.memset(arg4[:, :, :, 0:1], -100.0)
                dA = big.tile([128, 2304], bf16, name="sc_dA")
                ei = nc.scalar.activation(dA[:], arg[:], AF.Exp)
                last_exp[0] = ei

                # w = dtu (bcast n) * B (bcast d)
                wt = big.tile([128, 2304], bf16, name="sc_w")
                dtu3 = dtu[:].rearrange("p (d c) -> p d c", d=HEAD)
                B3 = B_t[k][:].rearrange("p (n c) -> p n c", n=NST)
                nc.vector.tensor_tensor(
                    wt[:].rearrange("p (d n c) -> p d n c", d=HEAD, n=NST),
                    dtu3.unsqueeze(2).broadcast_to([128, HEAD, NST, CI]),
                    B3.unsqueeze(1).broadcast_to([128, HEAD, NST, CI]),
                    Alu.mult)

                # scan along ci (fp32 state)
                hs = big.tile([128, 2304], bf16, name="sc_h")
                nc.vector.tensor_tensor_scan(hs[:], dA[:], wt[:], 0.0,
                                             Alu.mult, Alu.add)

                # y = sum_n h * C  (f32 tree)
                prod = big.tile([128, 2304], bf16, name="sc_prod")
                C3 = C_t[k][:].rearrange("p (n c) -> p n c", n=NST)
                nc.vector.tensor_tensor(
                    prod[:].rearrange("p (d n c) -> p d n c", d=HEAD, n=NST),
                    hs[:].rearrange("p (d n c) -> p d n c", d=HEAD, n=NST),
                    C3.unsqueeze(1).broadcast_to([128, HEAD, NST, CI]),
                    Alu.mult)
                pr4 = prod[:].rearrange("p (d n c) -> p d n c", d=HEAD, n=NST)
                s6 = big.tile([128, 1152], f32, name="sc_s6")
                s6v = s6[:].rearrange("p (d n c) -> p d n c", d=HEAD, n=6)
                nc.vector.tensor_tensor(s6v, pr4[:, :, 0:6], pr4[:, :, 6:12],
                                        Alu.add)
                s3 = big.tile([128, 576], f32, name="sc_s3")
                s3v = s3[:].rearrange("p (d n c) -> p d n c", d=HEAD, n=3)
                nc.vector.tensor_tensor(s3v, s6v[:, :, 0:3], s6v[:, :, 3:6],
                                        Alu.add)
                y0 = work.tile([128, 192], f32, name="sc_y0")
                y0v = y0[:].rearrange("p (d c) -> p d c", d=HEAD)
                nc.vector.tensor_tensor(y0v, s3v[:, :, 0], s3v[:, :, 1],
                                        Alu.add)
                y1 = work.tile([128, 192], f32, name="sc_y1")
                y1v = y1[:].rearrange("p (d c) -> p d c", d=HEAD)
                nc.vector.tensor_tensor(y1v, y0v, s3v[:, :, 2], Alu.add)
                dres = work.tile([128, 192], bf16, name="sc_dres")
                nc.vector.tensor_tensor(
                    dres[:].rearrange("p (d c) -> p d c", d=HEAD), u_dc,
                    cs['Ds_rep'][:, kd].unsqueeze(2).broadcast_to(
                        [128, HEAD, CI]), Alu.mult)
                y2 = work.tile([128, 192], f32, name="sc_y2")
                nc.vector.tensor_tensor(y2[:], y1[:], dres[:], Alu.add)

                # head-LN over ci (eps 1e-5) + affine
                y24 = y2[:].rearrange("p (d c) -> p d c", d=HEAD)
                mu = work.tile([128, 16], f32, name="sc_mu")
                nc.vector.tensor_reduce(mu[:], y24, AX.X, Alu.add)
                sq = work.tile([128, 192], f32, name="sc_sq")
                nc.scalar.activation(sq[:], y2[:], AF.Square)
                s2_ = work.tile([128, 16], f32, name="sc_s2")
                nc.vector.tensor_reduce(
                    s2_[:], sq[:].rearrange("p (d c) -> p d c", d=HEAD),
                    AX.X, Alu.add)
                nc.vector.tensor_scalar(mu[:], mu[:], 1.0 / CI, None, Alu.mult)
                m2 = work.tile([128, 16], f32, name="sc_m2")
                nc.vector.tensor_tensor(m2[:], mu[:], mu[:], Alu.mult)
                var = work.tile([128, 16], f32, name="sc_var")
                nc.vector.scalar_tensor_tensor(var[:], s2_[:], 1.0 / CI,
                                               m2[:], Alu.mult, Alu.subtract)
                lnv = work.tile([128, 16], f32, name="sc_lnv")
                nc.scalar.activation(lnv[:], var[:], AF.Ln, bias=eps5[:])
                rstd = work.tile([128, 16], f32, name="sc_rstd")
                nc.scalar.activation(rstd[:], lnv[:], AF.Exp, scale=-0.5)
                yc = work.tile([128, 192], bf16, name="sc_yc")
                yc3 = yc[:].rearrange("p (d c) -> p d c", d=HEAD)
                nc.vector.tensor_tensor(
                    yc3, y24,
                    mu[:].unsqueeze(2).broadcast_to([128, HEAD, CI]),
                    Alu.subtract)
                yn = work.tile([128, 192], bf16, name="sc_yn")
                yn3 = yn[:].rearrange("p (d c) -> p d c", d=HEAD)
                nc.vector.tensor_tensor(
                    yn3, yc3,
                    rstd[:].unsqueeze(2).broadcast_to([128, HEAD, CI]),
                    Alu.mult)
                ya = work.tile([128, 192], bf16, name="sc_ya")
                ya3 = ya[:].rearrange("p (d c) -> p d c", d=HEAD)
                nc.vector.tensor_tensor(
                    ya3, yn3,
                    cs['onw'][:, k * 12:(k + 1) * 12].unsqueeze(1)
                    .broadcast_to([128, HEAD, CI]), Alu.mult)
                yl = work.tile([128, 192], bf16, name="sc_yl")
                nc.vector.tensor_tensor(
                    yl[:].rearrange("p (d c) -> p d c", d=HEAD), ya3,
                    cs['onb'][:, k * 12:(k + 1) * 12].unsqueeze(1)
                    .broadcast_to([128, HEAD, CI]), Alu.add)

                # back to feature layout + outproj + residual
                ylf = {}
                for h in range(2):
                    pty = ps_tr.tile([96, 128], bf16, name="sc_psy", tag="ptr")
                    nc.tensor.matmul(pty[:, :],
                                     yl[:, h * 96:(h + 1) * 96],
                                     ident[:, :], is_transpose=True)
                    ylf[h] = work.tile([96, 128], bf16, name=f"sc_ylf{h}")
                    nc.scalar.activation(ylf[h][:], pty[:, :], AF.Copy)
                pz_ = ps_m2.tile([96, 128], f32, name="sc_pz", tag="pm2")
                nc.tensor.matmul(pz_[:, :],
                                 cs['wo'][:, (k * 2) * 96:(k * 2 + 1) * 96],
                                 ylf[0][:], start=True, stop=False)
                nc.tensor.matmul(pz_[:, :],
                                 cs['wo'][:, (k * 2 + 1) * 96:(k * 2 + 2) * 96],
                                 ylf[1][:], start=False, stop=True)
                nc.vector.tensor_tensor(z1s[k][:, p0:p0 + W], pz_[:, :],
                                        xr[k][:, sp0:sp0 + W], Alu.add)

                # LN3 (eps 1e-6) + affine
                pt3 = ps_tr.tile([128, 96], bf16, name="sc_ps3", tag="ptr")
                nc.tensor.matmul(pt3[:, :], z1s[k][:, p0:p0 + W],
                                 ident[:96, :96], is_transpose=True)
                zt = work.tile([128, 96], bf16, name="sc_zt")
                t1 = work.tile([128, 1], f32, name="sc_t1")
                nc.scalar.activation(zt[:], pt3[:, :], AF.Copy,
                                     accum_out=t1[:])
                zq = work.tile([128, 96], bf16, name="sc_zq")
                t2 = work.tile([128, 1], f32, name="sc_t2")
                nc.scalar.activation(zq[:], zt[:], AF.Square, accum_out=t2[:])
                mu3 = work.tile([128, 1], f32, name="sc_mu3")
                nc.vector.tensor_scalar(mu3[:], t1[:], 1.0 / 96, None,
                                        Alu.mult)
                m23 = work.tile([128, 1], f32, name="sc_m23")
                nc.vector.tensor_tensor(m23[:], mu3[:], mu3[:], Alu.mult)
                v3 = work.tile([128, 1], f32, name="sc_v3")
                nc.vector.scalar_tensor_tensor(v3[:], t2[:], 1.0 / 96, m23[:],
                                               Alu.mult, Alu.subtract)
                l3 = work.tile([128, 1], f32, name="sc_l3")
                nc.scalar.activation(l3[:], v3[:], AF.Ln, bias=eps6[:])
                r3 = work.tile([128, 1], f32, name="sc_r3")
                ei2 = nc.scalar.activation(r3[:], l3[:], AF.Exp, scale=-0.5)
                last_exp[0] = ei2
                zn_ = work.tile([128, 96], bf16, name="sc_zn")
                nc.vector.tensor_scalar(zn_[:], zt[:], mu3[:], r3[:],
                                        Alu.subtract, Alu.mult)
                za = work.tile([128, 96], bf16, name="sc_za")
                nc.vector.tensor_tensor(za[:], zn_[:],
                                        cs['nw'][:, k * 96:(k + 1) * 96],
                                        Alu.mult)
                zb = work.tile([128, 96], bf16, name="sc_zb")
                nc.vector.tensor_tensor(zb[:], za[:],
                                        cs['nb'][:, k * 96:(k + 1) * 96],
                                        Alu.add)
                pb3 = ps_tr.tile([96, 128], bf16, name="sc_pb3", tag="ptr")
                nc.tensor.matmul(pb3[:, :], zb[:, :], ident[:, :],
                                 is_transpose=True)
                nc.scalar.activation(zns[k][:, p0:p0 + W], pb3[:, :], AF.Copy)

        # ================= stage F: MLP (gelu table set) =================
        for r_ in range(ROWS):
            p0 = r_ * W
            for k in range(2):
                hid = {}
                for jj in range(3):
                    ph = ps_mm.tile([128, 128], f32, name="mlp_ph", tag="pmm")
                    nc.tensor.matmul(ph[:, :],
                                     cs['w1'][:, k * 384 + jj * 128:
                                              k * 384 + (jj + 1) * 128],
                                     zns[k][:, p0:p0 + W])
                    hid[jj] = work.tile([128, 128], bf16, name=f"mlp_hid{jj}")
                    if sim_safe:
                        # simulator lacks Gelu: sigmoid-approx for validation
                        sg = work.tile([128, 128], bf16, name="mlp_sg")
                        nc.scalar.activation(sg[:], ph[:, :], AF.Sigmoid,
                                             scale=1.702,
                                             bias=b1s[:, k * 3 + jj:
                                                      k * 3 + jj + 1])
                        vv = work.tile([128, 128], bf16, name="mlp_vv")
                        nc.scalar.activation(vv[:], ph[:, :], AF.Identity,
                                             bias=cs['b1'][:, k * 3 + jj:
                                                           k * 3 + jj + 1])
                        gi = nc.vector.tensor_tensor(hid[jj][:], vv[:], sg[:],
                                                     Alu.mult)
                    else:
                        gi = nc.scalar.activation(hid[jj][:], ph[:, :], AF.Gelu,
                                                  bias=cs['b1'][:, k * 3 + jj:
                                                                k * 3 + jj + 1])
                        if first_gelu[0] is None:
                            first_gelu[0] = gi
                pz2 = ps_m2.tile([96, 128], f32, name="mlp_pz2", tag="pm2")
                for jj in range(3):
                    nc.tensor.matmul(pz2[:, :],
                                     cs['w2'][:, (k * 3 + jj) * 96:
                                              (k * 3 + jj + 1) * 96],
                                     hid[jj][:], start=(jj == 0),
                                     stop=(jj == 2))
                ot = work.tile([96, 128], f32, name="mlp_ot")
                nc.vector.scalar_tensor_tensor(ot[:], pz2[:, :],
                                               cs['b2'][:, k:k + 1],
                                               z1s[k][:, p0:p0 + W],
                                               Alu.add, Alu.add)
                nc.sync.dma_start(y_out[k][:, p0:p0 + W], ot[:])

        # pin ACT table-set order: all Exp/Ln work before the first Gelu
        if first_gelu[0] is not None and last_exp[0] is not None:
            try:
                add_dep_helper(first_gelu[0].ins, last_exp[0].ins, False,
                               "act table set ordering")
            except Exception:
                pass

    nc.compile()
    return nc


_TRACE = False
_LAST_RESULT = None
_PROG_CACHE = {}


def kernel(**inputs):
    from concourse.bass_utils import run_bass_kernel_spmd

    consts = _prep_consts(inputs)
    consts_cast = {}
    for k, v in consts.items():
        if k in _F32_CONSTS:
            consts_cast[k] = np.ascontiguousarray(v, np.float32)
        else:
            consts_cast[k] = np.ascontiguousarray(v).astype(_bf16)

    if 'hw' not in _PROG_CACHE:
        _PROG_CACHE['hw'] = _build_program()
    nc = _PROG_CACHE['hw']

    in_maps = []
    for core in range(NCORES):
        m = dict(consts_cast)
        m.update(_core_slabs(inputs, core))
        in_maps.append(m)

    res = run_bass_kernel_spmd(nc, in_maps, core_ids=list(range(NCORES)),
                               trace=_TRACE)
    global _LAST_RESULT
    _LAST_RESULT = res

    y_r = np.zeros((B, C, H, W), np.float32)
    y_e = np.zeros((B, C, H, W), np.float32)
    for core in range(NCORES):
        b, r0 = core // 4, (core % 4) * ROWS
        y_r[b, :, r0:r0 + ROWS, :] = res.results[core]['y_r'].reshape(C, ROWS, W)
        y_e[b, :, r0:r0 + ROWS, :] = res.results[core]['y_e'].reshape(C, ROWS, W)
    return (y_r, y_e)



# revision 3
# speedup vs baseline: 2.0207x; 2.0207x over previous
# Trainium2 Bass kernel for ChannelMambaFusionBlock (2-modal channel-scan
# mamba fusion block).  Self-contained: takes FULL inputs, shards across 8
# NeuronCores internally (data-parallel over batch x H-rows), returns FULL
# outputs (y_r, y_e).
#
# Sharding: core i handles batch b = i//4, H-rows [32*(i%4), 32*(i%4)+32).
# Each core receives a 34-row halo slab of both input images (bf16, x-padded
# to 130 cols) plus replicated weights; all intermediates stay in SBUF.
#
# Channel order on-chip is d-major (e' = d*12 + c == the natural E index),
# so pos-layout tiles have (d, c) / (n, c) innermost-contiguous views and
# the big DVE tensor_tensor ops run in 2x_1p mode.
#
# Per-core pipeline:
#   A. channel-LN       positions-on-partitions via PE transpose;
#                       stats via DVE bn_stats/bn_aggr straight from PSUM
#   B. gconv 1x1        dense 96->192 matmul (block-diag weights baked in)
#      dwconv 3x3       9 diagonal accumulating matmuls over the padded slab
#      silu             single native Silu activation from PSUM (+bias)
#   C. xproj            one accumulating matmul pair per modality emits
#                       [B(n,c) | dts | C(n,c) | u(d,c)] in pos layout
#   D. channel scan     per H-row, per modality: [128pos, (d,n,c)=2304]:
#                         arg = dt*A_full (2x), dA = exp(arg) (ACT)
#                         wt  = dtu*B (2x), hs = tensor_tensor_scan
#                         y   = sum_n hs*C (2x mult + bf16 add tree)
#   E. head-LN, outproj, residual, channel-LN (bn_stats path)
#   F. MLP (exact gelu) + residual
#
# ACT table sets are steered (natural_log_exp / silu / gelu only) so the
# whole program needs ~4 ACT_TABLE_LOADs instead of ~530.

import numpy as np
import ml_dtypes

HEAD, NST, CI = 16, 12, 12       # heads, d_state, d_inner per head (scan len)
B, C, H, W = 2, 96, 128, 128
G, E = 24, 192
ROWS = 32                        # H-rows per core
SLAB_R = ROWS + 2                # with halo rows
WPAD = W + 2                     # x-padded row width
SLAB = SLAB_R * WPAD             # 4420
P = ROWS * W                     # 4096 inner positions per core
NCORES = 8
DNC = HEAD * NST * CI            # 2304
XPW = 2 * NST * CI + CI + HEAD * CI   # 492 = B(144) + dts(12) + C(144) + u(192)

_bf16 = ml_dtypes.bfloat16

_F32_CONSTS = ('cb', 'b1', 'b2')


def _prep_consts(inp):
    f32 = np.float32
    out = {}
    mods = ['rgb', 'e']

    out['ident'] = np.eye(128, dtype=f32)

    # dense per-mod expanded weight matrices, E index e = d*12 + c (natural)
    Wd = {}
    for k, m in enumerate(mods):
        ip = np.asarray(inp[f'inproj_{m}_w'], f32)                 # (24,8,4)
        Wk = np.zeros((E, C), f32)
        for g in range(G):
            Wk[g * 8:(g + 1) * 8, g * 4:(g + 1) * 4] = ip[g]
        Wd[k] = Wk

    # gconv lhsT [96, 2*192]: col k*192 + e, rows = input channel
    wi = np.zeros((96, 2 * 192), f32)
    for k in range(2):
        wi[:, k * 192:(k + 1) * 192] = Wd[k].T
    out['wi'] = wi

    # dwconv diag lhsT [96, 36*96] (tile q=(k,h), tap): chan j of tile q is
    # e = 96h + j
    convw = [np.asarray(inp[f'conv_{m}_w'], f32)[:, 0] for m in mods]
    convb = [np.asarray(inp[f'conv_{m}_b'], f32) for m in mods]
    wdw = np.zeros((96, 36 * 96), f32)
    cb = np.zeros((96, 4), f32)
    for q in range(4):
        k, h = q // 2, q % 2
        for j in range(96):
            e = 96 * h + j
            cb[j, q] = convb[k][e]
            for tap in range(9):
                dy, dx = tap // 3, tap % 3
                wdw[j, (q * 9 + tap) * 96 + j] = convw[k][e, dy, dx]
    out['wdw'] = wdw
    out['cb'] = cb

    # fused xproj rhs [96, (k,h)*492]:
    #   rows    = u-tile partition j = dloc*12 + c   (d = 8h + dloc)
    #   cols    0..143   B block   (n*12 + c)
    #           144..155 dts block (c)
    #           156..299 C block   (n*12 + c)
    #           300..491 u perm    (d*12 + c)
    xpw = np.asarray(inp['xproj_w'], f32)                          # (2,25,16)
    xpc = np.zeros((96, 4 * XPW), f32)
    for k in range(2):
        for h in range(2):
            o = (k * 2 + h) * XPW
            for dloc in range(8):
                d = 8 * h + dloc
                for c in range(CI):
                    row = dloc * 12 + c
                    for n in range(NST):
                        xpc[row, o + n * 12 + c] = xpw[k, 1 + n, d]
                        xpc[row, o + 156 + n * 12 + c] = xpw[k, 13 + n, d]
                    xpc[row, o + 144 + c] = xpw[k, 0, d]
                    xpc[row, o + 300 + d * 12 + c] = 1.0
    out['xpc'] = xpc

    rep = lambda v: np.broadcast_to(np.asarray(v, f32).reshape(1, -1),
                                    (128, np.asarray(v).size)).copy()

    # dense replicated scan constants, (d, c) layout per mod
    A = -np.exp(np.asarray(inp['A_logs'], f32))                    # (2,16,12)
    af = np.zeros((2, HEAD, NST, CI), f32)
    for k in range(2):
        af[k] = A[k][:, :, None]
    out['A_full'] = rep(af.reshape(2 * DNC))                       # [128,4608]
    dtw = np.asarray(inp['dtproj_w'], f32)[:, :, 0]                # (2,16)
    out['dtw_full'] = rep(np.repeat(dtw, CI, axis=1).reshape(-1))  # [128,384]
    dtb = np.asarray(inp['dtproj_b'], f32)
    out['dtb_full'] = rep(np.repeat(dtb, CI, axis=1).reshape(-1))
    Ds = np.asarray(inp['Ds'], f32)
    out['Ds_full'] = rep(np.repeat(Ds, CI, axis=1).reshape(-1))

    # affine params (used only when non-trivial; flags picked at build time)
    out['ga'] = rep(np.stack([np.asarray(inp['in1_w']), np.asarray(inp['in2_w'])]))
    out['be'] = rep(np.stack([np.asarray(inp['in1_b']), np.asarray(inp['in2_b'])]))
    out['nw'] = rep(np.stack([np.asarray(inp['n1_w']), np.asarray(inp['n2_w'])]))
    out['nb'] = rep(np.stack([np.asarray(inp['n1_b']), np.asarray(inp['n2_b'])]))
    onw = np.zeros((2, HEAD, CI), f32)
    onb = np.zeros((2, HEAD, CI), f32)
    for k, nm in enumerate(['on1', 'on2']):
        onw[k, :, :] = np.asarray(inp[f'{nm}_w'], f32)[None, :]
        onb[k, :, :] = np.asarray(inp[f'{nm}_b'], f32)[None, :]
    out['onw_full'] = rep(onw.reshape(-1))                         # [128,384]
    out['onb_full'] = rep(onb.reshape(-1))

    # outproj lhsT [96, 4*96]: block (k,h): rows = e - 96h, cols = C
    wo = np.zeros((96, 4 * 96), f32)
    for k, m in enumerate(mods):
        op = np.asarray(inp[f'outproj_{m}_w'], f32)                # (24,4,8)
        Wo = np.zeros((C, E), f32)
        for g in range(G):
            Wo[g * 4:(g + 1) * 4, g * 8:(g + 1) * 8] = op[g]
        for h in range(2):
            wo[:, (k * 2 + h) * 96:(k * 2 + h + 1) * 96] = \
                Wo[:, h * 96:(h + 1) * 96].T
    out['wo'] = wo

    # mlp
    w1 = np.zeros((96, 2 * 384), f32)
    w2 = np.zeros((128, 6 * 96), f32)
    b1 = np.zeros((128, 6), f32)
    b2 = np.zeros((96, 2), f32)
    for k, m in enumerate(mods):
        w1[:, k * 384:(k + 1) * 384] = np.asarray(inp[f'mlp_{m}_fc1_w'], f32).T
        b1k = np.asarray(inp[f'mlp_{m}_fc1_b'], f32)
        w2k = np.asarray(inp[f'mlp_{m}_fc2_w'], f32)               # (96, 384)
        for j in range(3):
            w2[:, (k * 3 + j) * 96:(k * 3 + j + 1) * 96] = \
                w2k[:, j * 128:(j + 1) * 128].T
            b1[:, k * 3 + j] = b1k[j * 128:(j + 1) * 128]
        b2[:, k] = np.asarray(inp[f'mlp_{m}_fc2_b'], f32)
    out['w1'], out['w2'], out['b1'], out['b2'] = w1, w2, b1, b2
    return out


def _affine_flags(inp):
    triv = lambda w, b: bool(np.all(np.asarray(inp[w]) == 1.0)
                             and np.all(np.asarray(inp[b]) == 0.0))
    return (triv('in1_w', 'in1_b') and triv('in2_w', 'in2_b'),
            triv('on1_w', 'on1_b') and triv('on2_w', 'on2_b'),
            triv('n1_w', 'n1_b') and triv('n2_w', 'n2_b'))


def _core_slabs(inp, core):
    b, r0 = core // 4, (core % 4) * ROWS
    res = {}
    for name, key in (('x_r', 'x_rgb'), ('x_e', 'x_e')):
        x = np.asarray(inp[key], np.float32)
        s = np.zeros((C, SLAB_R, WPAD), np.float32)
        lo, hi = r0 - 1, r0 + ROWS + 1
        slo, shi = max(lo, 0), min(hi, H)
        s[:, (slo - lo):(shi - lo), 1:W + 1] = x[b, :, slo:shi, :]
        res[name] = np.ascontiguousarray(s.reshape(C, SLAB)).astype(_bf16)
    return res


# Only these ACT table sets stay selectable; their act_info.json positions
# are preserved (other sets get an empty function list) so the emitted
# act_func_set_id values still index the real file.
_ACT_SETS_KEEP = ('natural_log_exp_and_others', 'silu_and_others',
                  'gelu_and_others')


def _build_program(flags):
    import concourse.mybir as mybir
    from concourse import bacc
    import concourse.tile as tile
    from contextlib import ExitStack

    triv_in, triv_on, triv_n = flags

    bf16 = mybir.dt.bfloat16
    f32 = mybir.dt.float32
    Alu = mybir.AluOpType
    AF = mybir.ActivationFunctionType
    AX = mybir.AxisListType

    nc = bacc.Bacc("TRN2", target_bir_lowering=False, debug=False)

    shapes = {
        'x_r': (C, SLAB), 'x_e': (C, SLAB), 'ident': (128, 128),
        'wi': (96, 384), 'wdw': (96, 36 * 96), 'cb': (96, 4),
        'xpc': (96, 4 * XPW),
        'A_full': (128, 2 * DNC), 'dtw_full': (128, 384),
        'dtb_full': (128, 384), 'Ds_full': (128, 384),
        'wo': (96, 384), 'w1': (96, 768), 'w2': (128, 576),
        'b1': (128, 6), 'b2': (96, 2),
    }
    if not triv_in:
        shapes['ga'] = (128, 192)
        shapes['be'] = (128, 192)
    if not triv_on:
        shapes['onw_full'] = (128, 384)
        shapes['onb_full'] = (128, 384)
    if not triv_n:
        shapes['nw'] = (128, 192)
        shapes['nb'] = (128, 192)

    dram = {}
    for name, shp in shapes.items():
        dt = f32 if name in _F32_CONSTS else bf16
        dram[name] = nc.dram_tensor(name, list(shp), dt,
                                    kind="ExternalInput").ap()
    y_out = {0: nc.dram_tensor('y_r', [C, P], f32, kind="ExternalOutput").ap(),
             1: nc.dram_tensor('y_e', [C, P], f32, kind="ExternalOutput").ap()}

    with tile.TileContext(nc) as tc, ExitStack() as ctx:
        slab = ctx.enter_context(tc.tile_pool(name="slab", bufs=1))
        cst = ctx.enter_context(tc.tile_pool(name="cst", bufs=1))
        big = ctx.enter_context(tc.tile_pool(name="big", bufs=1))
        work = ctx.enter_context(tc.tile_pool(name="work", bufs=2))
        ps_tr = ctx.enter_context(tc.tile_pool(name="ps_tr", bufs=2, space="PSUM"))
        ps_mm = ctx.enter_context(tc.tile_pool(name="ps_mm", bufs=2, space="PSUM"))
        ps_xp = ctx.enter_context(tc.tile_pool(name="ps_xp", bufs=2, space="PSUM"))
        ps_m2 = ctx.enter_context(tc.tile_pool(name="ps_m2", bufs=2, space="PSUM"))

        cs = {}
        for name in shapes:
            if name in ('x_r', 'x_e'):
                continue
            dt = f32 if name in _F32_CONSTS else bf16
            cs[name] = cst.tile(list(shapes[name]), dt, name=f"c_{name}")
            nc.sync.dma_start(cs[name][:], dram[name])
        ident = cs['ident']
        eps6 = cst.tile([128, 1], f32, name="eps6")
        nc.vector.memset(eps6[:], 1e-6)
        eps5 = cst.tile([128, 1], f32, name="eps5")
        nc.vector.memset(eps5[:], 1e-5)

        xs = {0: slab.tile([C, SLAB], bf16, name="xs_r"),
              1: slab.tile([C, SLAB], bf16, name="xs_e")}
        nc.sync.dma_start(xs[0][:], dram['x_r'])
        nc.sync.dma_start(xs[1][:], dram['x_e'])

        # ================= stage A: channel LN (eps 1e-6) =================
        xr = {0: slab.tile([C, SLAB], bf16, name="xr_r"),
              1: slab.tile([C, SLAB], bf16, name="xr_e")}
        nchunkA = (SLAB + 127) // 128
        for k in range(2):
            for j in range(nchunkA):
                p0 = j * 128
                npos = min(128, SLAB - p0)
                pt = ps_tr.tile([128, 96], bf16, name="lnA_ps", tag="ptr")
                nc.tensor.matmul(pt[:npos, :], xs[k][:, p0:p0 + npos],
                                 ident[:96, :96], is_transpose=True)
                st6 = work.tile([128, 6], f32, name="lnA_st6")
                nc.vector.bn_stats(st6[:npos], pt[:npos, :])
                mv = work.tile([128, 2], f32, name="lnA_mv")
                nc.vector.bn_aggr(mv[:npos], st6[:npos])
                lnv = work.tile([128, 1], f32, name="lnA_lnv")
                nc.scalar.activation(lnv[:npos], mv[:npos, 1:2], AF.Ln,
                                     bias=eps6[:npos])
                rstd = work.tile([128, 1], f32, name="lnA_rstd")
                nc.scalar.activation(rstd[:npos], lnv[:npos], AF.Exp,
                                     scale=-0.5)
                xn = work.tile([128, 96], bf16, name="lnA_xn")
                nc.vector.tensor_scalar(xn[:npos], pt[:npos, :],
                                        mv[:npos, 0:1], rstd[:npos],
                                        Alu.subtract, Alu.mult)
                if not triv_in:
                    xa = work.tile([128, 96], bf16, name="lnA_xa")
                    nc.vector.tensor_tensor(xa[:npos], xn[:npos],
                                            cs['ga'][:npos, k * 96:(k + 1) * 96],
                                            Alu.mult)
                    xn = work.tile([128, 96], bf16, name="lnA_xb")
                    nc.vector.tensor_tensor(xn[:npos], xa[:npos],
                                            cs['be'][:npos, k * 96:(k + 1) * 96],
                                            Alu.add)
                pb = ps_tr.tile([96, 128], bf16, name="lnA_pb", tag="ptr")
                nc.tensor.matmul(pb[:, :npos], xn[:npos, :],
                                 ident[:npos, :npos], is_transpose=True)
                nc.scalar.activation(xr[k][:, p0:p0 + npos], pb[:, :npos],
                                     AF.Copy)

        # ================= stage B: gconv =================
        z = {q: slab.tile([96, SLAB], bf16, name=f"z{q}") for q in range(4)}
        CH = 512
        nchunkB = (SLAB + CH - 1) // CH
        for j in range(nchunkB):
            p0 = j * CH
            npos = min(CH, SLAB - p0)
            for q in range(4):
                k, h = q // 2, q % 2
                pz = ps_mm.tile([96, CH], f32, name="gc_ps", tag="pmm")
                nc.tensor.matmul(pz[:, :npos],
                                 cs['wi'][:, k * 192 + h * 96:
                                          k * 192 + (h + 1) * 96],
                                 xr[k][:, p0:p0 + npos])
                nc.scalar.activation(z[q][:, p0:p0 + npos], pz[:, :npos],
                                     AF.Copy)

        # ========= stage B2: dwconv (9 diag taps) + bias + silu ==========
        u = {q: slab.tile([96, SLAB], bf16, name=f"u{q}") for q in range(4)}
        for q in range(4):
            for st in range(131, 4289, 512):
                npos = min(512, 4289 - st)
                pu = ps_mm.tile([96, 512], f32, name="dw_ps", tag="pmm")
                for tap in range(9):
                    dy, dx = tap // 3, tap % 3
                    dlt = (dy - 1) * WPAD + (dx - 1)
                    nc.tensor.matmul(
                        pu[:, :npos],
                        cs['wdw'][:, (q * 9 + tap) * 96:(q * 9 + tap + 1) * 96],
                        z[q][:, st + dlt: st + dlt + npos],
                        start=(tap == 0), stop=(tap == 8))
                nc.scalar.activation(u[q][:, st:st + npos], pu[:, :npos],
                                     AF.Silu, bias=cs['cb'][:, q:q + 1])

        # ============ stages C/D/E: per-row scan pipeline ============
        z1s = {0: slab.tile([96, P], bf16, name="z1_r"),
               1: slab.tile([96, P], bf16, name="z1_e")}
        zns = {0: slab.tile([96, P], bf16, name="zn_r", tag="xs_r"),
               1: slab.tile([96, P], bf16, name="zn_e", tag="xs_e")}

        for r_ in range(ROWS):
            p0 = r_ * W
            sp0 = (r_ + 1) * WPAD + 1
            # ---- fused xproj: sc[k] = [B(144) | dts(12) | C(144) | u(192)]
            sc = {}
            for k in range(2):
                pxp = ps_xp.tile([128, XPW], f32, name="xp_ps", tag="pxp")
                for h in range(2):
                    o = (k * 2 + h) * XPW
                    nc.tensor.matmul(pxp[:, :],
                                     u[k * 2 + h][:, sp0:sp0 + W],
                                     cs['xpc'][:, o:o + XPW],
                                     start=(h == 0), stop=(h == 1))
                sc[k] = work.tile([128, XPW], bf16, name=f"sc{k}")
                nc.scalar.activation(sc[k][:], pxp[:, :], AF.Copy)

            # ---- per-modality scan core ----
            for k in range(2):
                ks = slice(k * 192, (k + 1) * 192)
                dts2 = sc[k][:, 144:156]
                u_dc = sc[k][:, 300:492]
                # dt = softplus(dts*dtw + dtb)   [(d,c) = 192]
                pre = work.tile([128, 192], bf16, name="sc_pre")
                pre3 = pre[:].rearrange("p (d c) -> p d c", d=HEAD)
                nc.vector.tensor_tensor(
                    pre3,
                    dts2.unsqueeze(1).broadcast_to([128, HEAD, CI]),
                    cs['dtw_full'][:, ks].rearrange("p (d c) -> p d c", d=HEAD),
                    Alu.mult)
                pre2 = work.tile([128, 192], bf16, name="sc_pre2")
                nc.vector.tensor_tensor(pre2[:], pre[:],
                                        cs['dtb_full'][:, ks], Alu.add)
                ab = work.tile([128, 192], bf16, name="sc_ab")
                nc.scalar.activation(ab[:], pre2[:], AF.Abs)
                exn = work.tile([128, 192], bf16, name="sc_exn")
                nc.scalar.activation(exn[:], ab[:], AF.Exp, scale=-1.0)
                lg = work.tile([128, 192], bf16, name="sc_lg")
                nc.scalar.activation(lg[:], exn[:], AF.Ln, bias=1.0)
                rl = work.tile([128, 192], bf16, name="sc_rl")
                nc.vector.tensor_scalar(rl[:], pre2[:], 0.0, None, Alu.max)
                dt = work.tile([128, 192], bf16, name="sc_dt")
                nc.vector.tensor_tensor(dt[:], rl[:], lg[:], Alu.add)
                dt3 = dt[:].rearrange("p (d c) -> p d c", d=HEAD)

                # dtu  [(d,c)]
                dtu = work.tile([128, 192], bf16, name="sc_dtu")
                nc.vector.tensor_tensor(dtu[:], dt[:], u_dc, Alu.mult)

                # dA = exp(dt * A); c=0 forced to ~0 (scan segment reset)
                arg = big.tile([128, DNC], bf16, name="sc_arg")
                arg4 = arg[:].rearrange("p (d n c) -> p d n c", d=HEAD, n=NST)
                nc.vector.tensor_tensor(
                    arg4,
                    dt3.unsqueeze(2).broadcast_to([128, HEAD, NST, CI]),
                    cs['A_full'][:, k * DNC:(k + 1) * DNC].rearrange(
                        "p (d n c) -> p d n c", d=HEAD, n=NST),
                    Alu.mult)
                nc.vector.memset(arg4[:, :, :, 0:1], -100.0)
                dA = big.tile([128, DNC], bf16, name="sc_dA")
                nc.scalar.activation(dA[:], arg[:], AF.Exp)

                # w = dtu (bcast n) * B (bcast d)
                wt = big.tile([128, DNC], bf16, name="sc_w")
                B4 = sc[k][:, 0:144].rearrange("p (n c) -> p n c", n=NST)
                nc.vector.tensor_tensor(
                    wt[:].rearrange("p (d n c) -> p d n c", d=HEAD, n=NST),
                    dtu[:].rearrange("p (d c) -> p d c", d=HEAD)
                    .unsqueeze(2).broadcast_to([128, HEAD, NST, CI]),
                    B4.unsqueeze(1).broadcast_to([128, HEAD, NST, CI]),
                    Alu.mult)

                # scan along c (fp32 state)
                hs = big.tile([128, DNC], bf16, name="sc_h")
                nc.vector.tensor_tensor_scan(hs[:], dA[:], wt[:], 0.0,
                                             Alu.mult, Alu.add)

                # y = sum_n h * C  (bf16 tree), C is cross-modal
                prod = big.tile([128, DNC], bf16, name="sc_prod")
                C4 = sc[1 - k][:, 156:300].rearrange("p (n c) -> p n c", n=NST)
                nc.vector.tensor_tensor(
                    prod[:].rearrange("p (d n c) -> p d n c", d=HEAD, n=NST),
                    hs[:].rearrange("p (d n c) -> p d n c", d=HEAD, n=NST),
                    C4.unsqueeze(1).broadcast_to([128, HEAD, NST, CI]),
                    Alu.mult)
                pr4 = prod[:].rearrange("p (d n c) -> p d n c", d=HEAD, n=NST)
                s6 = big.tile([128, HEAD * 6 * CI], bf16, name="sc_s6")
                s6v = s6[:].rearrange("p (d n c) -> p d n c", d=HEAD, n=6)
                nc.vector.tensor_tensor(s6v, pr4[:, :, 0:6], pr4[:, :, 6:12],
                                        Alu.add)
                s3 = big.tile([128, HEAD * 3 * CI], bf16, name="sc_s3")
                s3v = s3[:].rearrange("p (d n c) -> p d n c", d=HEAD, n=3)
                nc.vector.tensor_tensor(s3v, s6v[:, :, 0:3], s6v[:, :, 3:6],
                                        Alu.add)
                y0 = work.tile([128, 192], bf16, name="sc_y0")
                y0v = y0[:].rearrange("p (d c) -> p d c", d=HEAD)
                nc.vector.tensor_tensor(y0v, s3v[:, :, 0], s3v[:, :, 1],
                                        Alu.add)
                y1 = work.tile([128, 192], bf16, name="sc_y1")
                nc.vector.tensor_tensor(y1[:], y0[:], s3[:].rearrange(
                    "p (d n c) -> p d n c", d=HEAD, n=3)[:, :, 2], Alu.add)
                dres = work.tile([128, 192], bf16, name="sc_dres")
                nc.vector.tensor_tensor(dres[:], u_dc,
                                        cs['Ds_full'][:, ks], Alu.mult)
                y2 = work.tile([128, 192], f32, name="sc_y2")
                nc.vector.tensor_tensor(y2[:], y1[:], dres[:], Alu.add)

                # head-LN over c (eps 1e-5)
                y24 = y2[:].rearrange("p (d c) -> p d c", d=HEAD)
                mu = work.tile([128, 16], f32, name="sc_mu")
                nc.vector.tensor_reduce(mu[:], y24, AX.X, Alu.add)
                sq = work.tile([128, 192], bf16, name="sc_sq")
                nc.scalar.activation(sq[:], y2[:], AF.Square)
                s2_ = work.tile([128, 16], f32, name="sc_s2")
                nc.vector.tensor_reduce(
                    s2_[:], sq[:].rearrange("p (d c) -> p d c", d=HEAD),
                    AX.X, Alu.add)
                nc.vector.tensor_scalar(mu[:], mu[:], 1.0 / CI, None, Alu.mult)
                m2 = work.tile([128, 16], f32, name="sc_m2")
                nc.vector.tensor_tensor(m2[:], mu[:], mu[:], Alu.mult)
                var = work.tile([128, 16], f32, name="sc_var")
                nc.vector.scalar_tensor_tensor(var[:], s2_[:], 1.0 / CI,
                                               m2[:], Alu.mult, Alu.subtract)
                lnv = work.tile([128, 16], f32, name="sc_lnv")
                nc.scalar.activation(lnv[:], var[:], AF.Ln, bias=eps5[:])
                rstd = work.tile([128, 16], f32, name="sc_rstd")
                nc.scalar.activation(rstd[:], lnv[:], AF.Exp, scale=-0.5)
                yc = work.tile([128, 192], bf16, name="sc_yc")
                yc3 = yc[:].rearrange("p (d c) -> p d c", d=HEAD)
                nc.vector.tensor_tensor(
                    yc3, y24,
                    mu[:].unsqueeze(2).broadcast_to([128, HEAD, CI]),
                    Alu.subtract)
                yl = work.tile([128, 192], bf16, name="sc_yn")
                yl3 = yl[:].rearrange("p (d c) -> p d c", d=HEAD)
                nc.vector.tensor_tensor(
                    yl3, yc3,
                    rstd[:].unsqueeze(2).broadcast_to([128, HEAD, CI]),
                    Alu.mult)
                if not triv_on:
                    ya = work.tile([128, 192], bf16, name="sc_ya")
                    nc.vector.tensor_tensor(ya[:], yl[:],
                                            cs['onw_full'][:, ks], Alu.mult)
                    yl = work.tile([128, 192], bf16, name="sc_yl")
                    nc.vector.tensor_tensor(yl[:], ya[:],
                                            cs['onb_full'][:, ks], Alu.add)

                # back to feature layout + outproj + residual
                ylf = {}
                for h in range(2):
                    pty = ps_tr.tile([96, 128], bf16, name="sc_psy", tag="ptr")
                    nc.tensor.matmul(pty[:, :],
                                     yl[:, h * 96:(h + 1) * 96],
                                     ident[:, :], is_transpose=True)
                    ylf[h] = work.tile([96, 128], bf16, name=f"sc_ylf{h}")
                    nc.scalar.activation(ylf[h][:], pty[:, :], AF.Copy)
                pz_ = ps_m2.tile([96, 128], f32, name="sc_pz", tag="pm2")
                nc.tensor.matmul(pz_[:, :],
                                 cs['wo'][:, (k * 2) * 96:(k * 2 + 1) * 96],
                                 ylf[0][:], start=True, stop=False)
                nc.tensor.matmul(pz_[:, :],
                                 cs['wo'][:, (k * 2 + 1) * 96:(k * 2 + 2) * 96],
                                 ylf[1][:], start=False, stop=True)
                nc.vector.tensor_tensor(z1s[k][:, p0:p0 + W], pz_[:, :],
                                        xr[k][:, sp0:sp0 + W], Alu.add)

                # LN3 (eps 1e-6)
                pt3 = ps_tr.tile([128, 96], bf16, name="sc_ps3", tag="ptr")
                nc.tensor.matmul(pt3[:, :], z1s[k][:, p0:p0 + W],
                                 ident[:96, :96], is_transpose=True)
                st3 = work.tile([128, 6], f32, name="sc_st3")
                nc.vector.bn_stats(st3[:], pt3[:, :])
                mv3 = work.tile([128, 2], f32, name="sc_mv3")
                nc.vector.bn_aggr(mv3[:], st3[:])
                l3 = work.tile([128, 1], f32, name="sc_l3")
                nc.scalar.activation(l3[:], mv3[:, 1:2], AF.Ln, bias=eps6[:])
                r3 = work.tile([128, 1], f32, name="sc_r3")
                nc.scalar.activation(r3[:], l3[:], AF.Exp, scale=-0.5)
                zn_ = work.tile([128, 96], bf16, name="sc_zn")
                nc.vector.tensor_scalar(zn_[:], pt3[:, :], mv3[:, 0:1], r3[:],
                                        Alu.subtract, Alu.mult)
                if not triv_n:
                    za = work.tile([128, 96], bf16, name="sc_za")
                    nc.vector.tensor_tensor(za[:], zn_[:],
                                            cs['nw'][:, k * 96:(k + 1) * 96],
                                            Alu.mult)
                    zn_ = work.tile([128, 96], bf16, name="sc_zb")
                    nc.vector.tensor_tensor(zn_[:], za[:],
                                            cs['nb'][:, k * 96:(k + 1) * 96],
                                            Alu.add)
                pb3 = ps_tr.tile([96, 128], bf16, name="sc_pb3", tag="ptr")
                nc.tensor.matmul(pb3[:, :], zn_[:, :], ident[:, :],
                                 is_transpose=True)
                nc.scalar.activation(zns[k][:, p0:p0 + W], pb3[:, :], AF.Copy)

        # ================= stage F: MLP (gelu table set) =================
        for r_ in range(ROWS):
            p0 = r_ * W
            for k in range(2):
                hid = {}
                for jj in range(3):
                    ph = ps_mm.tile([128, 128], f32, name="mlp_ph", tag="pmm")
                    nc.tensor.matmul(ph[:, :],
                                     cs['w1'][:, k * 384 + jj * 128:
                                              k * 384 + (jj + 1) * 128],
                                     zns[k][:, p0:p0 + W])
                    hid[jj] = work.tile([128, 128], bf16, name=f"mlp_hid{jj}")
                    nc.scalar.activation(hid[jj][:], ph[:, :], AF.Gelu,
                                         bias=cs['b1'][:, k * 3 + jj:
                                                       k * 3 + jj + 1])
                pz2 = ps_m2.tile([96, 128], f32, name="mlp_pz2", tag="pm2")
                for jj in range(3):
                    nc.tensor.matmul(pz2[:, :],
                                     cs['w2'][:, (k * 3 + jj) * 96:
                                              (k * 3 + jj + 1) * 96],
                                     hid[jj][:], start=(jj == 0),
                                     stop=(jj == 2))
                ot = work.tile([96, 128], f32, name="mlp_ot")
                nc.vector.scalar_tensor_tensor(ot[:], pz2[:, :],
                                               cs['b2'][:, k:k + 1],
                                               z1s[k][:, p0:p0 + W],
                                               Alu.add, Alu.add)
                nc.sync.dma_start(y_out[k][:, p0:p0 + W], ot[:])

    # steer ACT table-set selection: only the kept sets are non-empty, so
    # ln/exp/abs/square/copy land in natural_log_exp_and_others, silu in
    # silu_and_others, gelu in gelu_and_others (positions preserved).
    orig = bacc.get_activation_tables

    def steered(arch):
        t = orig(arch)
        return {k: (v if k in _ACT_SETS_KEEP else set()) for k, v in t.items()}

    bacc.get_activation_tables = steered
    try:
        nc.compile()
    finally:
        bacc.get_activation_tables = orig
    return nc


_TRACE = False
_LAST_RESULT = None
_PROG_CACHE = {}


def kernel(**inputs):
    from concourse.bass_utils import run_bass_kernel_spmd

    flags = _affine_flags(inputs)
    consts = _prep_consts(inputs)
    triv_in, triv_on, triv_n = flags
    drop = []
    if triv_in:
        drop += ['ga', 'be']
    if triv_on:
        drop += ['onw_full', 'onb_full']
    if triv_n:
        drop += ['nw', 'nb']
    for d in drop:
        consts.pop(d, None)

    consts_cast = {}
    for k, v in consts.items():
        if k in _F32_CONSTS:
            consts_cast[k] = np.ascontiguousarray(v, np.float32)
        else:
            consts_cast[k] = np.ascontiguousarray(v).astype(_bf16)

    if flags not in _PROG_CACHE:
        _PROG_CACHE[flags] = _build_program(flags)
    nc = _PROG_CACHE[flags]

    in_maps = []
    for core in range(NCORES):
        m = dict(consts_cast)
        m.update(_core_slabs(inputs, core))
        in_maps.append(m)

    res = run_bass_kernel_spmd(nc, in_maps, core_ids=list(range(NCORES)),
                               trace=_TRACE)
    global _LAST_RESULT
    _LAST_RESULT = res

    y_r = np.zeros((B, C, H, W), np.float32)
    y_e = np.zeros((B, C, H, W), np.float32)
    for core in range(NCORES):
        b, r0 = core // 4, (core % 4) * ROWS
        y_r[b, :, r0:r0 + ROWS, :] = res.results[core]['y_r'].reshape(C, ROWS, W)
        y_e[b, :, r0:r0 + ROWS, :] = res.results[core]['y_e'].reshape(C, ROWS, W)
    return (y_r, y_e)
